# revision 8
# baseline (speedup 1.0000x reference)
"""FCCaps (capsule fully-connected layer with dynamic routing) on 8 TRN2 cores.

Sharding: input capsules N=1024 are split 128 per core (w1 is read exactly
once across the chip); batch B=32 is replicated on partitions. Per-core
layout puts partition p = 32*g + b (g = n%4 group, b = batch), free axes
(f = n//4, d, class) -- innermost axis is classes so DVE tensor_tensor ops
keep 2x mode under d-broadcasts. Routing reductions over n run on the PE
via a block-diagonal-ones stationary accumulating into PSUM; the tiny
[B,D(+count),C] partial sums are AllReduced across cores each iteration.
"""
import numpy as np

NCORES = 8
B, N, CI, C, D = 32, 1024, 128, 20, 64
NL = N // NCORES          # 128 local capsules
F, G = 32, 4              # n_local = 4*f + g
CD = C * D                # 1280
NUM_ITERS = 3

_prog = None


def _build():
    import os
    STAGE = int(os.environ.get("KERNEL_STAGE", "99"))
    import concourse.mybir as mybir
    import concourse.bacc as bacc
    import concourse.tile as tile

    f32 = mybir.dt.float32
    bf16 = mybir.dt.bfloat16
    DT_W = bf16     # w1/x dram + sbuf dtype
    DT_U = bf16     # u_hat / t / c / v_rep storage
    X = mybir.AxisListType.X
    ADD = mybir.AluOpType.add
    MULT = mybir.AluOpType.mult
    AF = mybir.ActivationFunctionType

    nc = bacc.Bacc("TRN2", target_bir_lowering=False, debug=False,
                   num_devices=NCORES)

    xT_d = nc.dram_tensor("xT", [CI, NL, B], DT_W, kind="ExternalInput").ap()
    w1t_d = nc.dram_tensor("w1t", [NL, CI, D, C], DT_W,
                           kind="ExternalInput").ap()
    blk_d = nc.dram_tensor("blk", [128, B], DT_U, kind="ExternalInput").ap()
    poses_d = nc.dram_tensor("poses", [B, D, C], f32,
                             kind="ExternalOutput").ap()
    acts_d = nc.dram_tensor("acts", [B, C], f32, kind="ExternalOutput").ap()
    dbg_d = None
    if STAGE < 99:
        dbg_d = nc.dram_tensor("dbg", [128, F, C], f32,
                               kind="ExternalOutput").ap()

    # d-row chunk splits (rows of C=20 fp32 words; PSUM bank = 512 words)
    RSPLIT = [(0, 25), (25, 50), (50, 64)]       # for [*, 64, 20] psum
    RSPLIT65 = [(0, 25), (25, 50), (50, 65)]     # for [*, 65, 20] psum

    with tile.TileContext(nc) as tc:
        with tc.tile_pool(name="persist", bufs=1) as persist, \
             tc.tile_pool(name="wpool", bufs=2) as wpool, \
             tc.tile_pool(name="sqpool", bufs=2) as sqpool, \
             tc.tile_pool(name="treep", bufs=1) as treep, \
             tc.tile_pool(name="smallp", bufs=1) as smallp, \
             tc.tile_pool(name="psA", bufs=1, space="PSUM") as psA, \
             tc.tile_pool(name="psS", bufs=1, space="PSUM") as psS, \
             tc.tile_pool(name="dramp", bufs=1, space="DRAM") as dramp:

            # ---- persistent tiles ----
            uh = persist.tile([128, F, D, C], DT_U)     # u_hat, bf16
            x_sb = persist.tile([CI, NL, B], DT_W)
            blk_sb = persist.tile([128, B], DT_U)
            u2 = persist.tile([128, F, C], f32)         # |u_hat|^2 over d
            A_t = persist.tile([128, F, C], f32)        # 1 - (u2/(.5+u2))^2
            G2_t = persist.tile([128, F, C], f32)       # 2*sqrt(u2)/(.5+u2)
            bij = persist.tile([128, F, C], f32)
            dot = persist.tile([128, F, C], f32)

            # collective buffers
            cc_in = [dramp.tile([B, D, C], f32, name="cc_in0")] + [
                dramp.tile([B, D + 1, C], f32, name=f"cc_in{i}")
                for i in (1, 2)]
            cc_out = [dramp.tile([B, D, C], f32, addr_space="Shared",
                                 name="cc_out0")] + [
                dramp.tile([B, D + 1, C], f32, addr_space="Shared",
                           name=f"cc_out{i}") for i in (1, 2)]

            nc.sync.dma_start(x_sb[:], xT_d[:])
            nc.sync.dma_start(blk_sb[:], blk_d[:])

            # s0 accumulators (c uniform in iter 0 -> plain sum over n)
            sp = [psS.tile([32, 25, C], f32, tag="spA", name="sp_a"),
                  psS.tile([32, 25, C], f32, tag="spB", name="sp_b"),
                  psS.tile([32, 15, C], f32, tag="spC", name="sp_c")]

            # ---- Phase A: u_hat = w1 @ x, norms, incremental s0 ----
            for q in range(F):
                w_t = wpool.tile([CI, G, D, C], DT_W, tag="big",
                                 name=f"w_t{q}")
                nc.sync.dma_start(
                    w_t[:, 0:2], w1t_d[4 * q:4 * q + 2]
                    .rearrange("n i d c -> i n d c"))
                nc.sync.dma_start(
                    w_t[:, 2:4], w1t_d[4 * q + 2:4 * q + 4]
                    .rearrange("n i d c -> i n d c"))
                ph = [psA.tile([128, r1 - r0, C], f32, tag=f"psA{k}",
                               name=f"ph{k}_{q}")
                      for k, (r0, r1) in enumerate(RSPLIT)]
                for g in range(G):
                    lhsT = x_sb[:, 4 * q + g, :]
                    for k, (r0, r1) in enumerate(RSPLIT):
                        nc.tensor.matmul(
                            ph[k][32 * g:32 * g + 32, :, :],
                            lhsT=lhsT,
                            rhs=w_t[:, g, r0:r1, :],
                            start=True, stop=True,
                            tile_position=(0, 32 * g))
                sq = sqpool.tile([128, D, C], f32, tag="sq", name=f"sq{q}")
                for k, (r0, r1) in enumerate(RSPLIT):
                    nc.scalar.activation(uh[:, q, r0:r1, :], ph[k][:],
                                         AF.Copy)
                    nc.scalar.square(sq[:, r0:r1, :], ph[k][:])
                nc.vector.tensor_reduce(u2[:, q], sq.transpose([0, 2, 1]),
                                        X, ADD)
                for spt, (r0, r1) in zip(sp, RSPLIT):
                    nc.tensor.matmul(spt[:, 0:r1 - r0, :], lhsT=blk_sb[:],
                                     rhs=uh[:, q, r0:r1, :],
                                     start=(q == 0), stop=(q == F - 1))

            if STAGE <= 1:
                nc.sync.dma_start(dbg_d[:], u2[:])
                z32 = smallp.tile([32, D, C], f32, tag="v32")
                nc.vector.tensor_scalar_add(z32[:], z32[:], 0.0)
                za = smallp.tile([32, C], f32, tag="a32o")
                nc.vector.tensor_scalar_add(za[:], za[:], 0.0)
                nc.sync.dma_start(poses_d[:], z32[:])
                nc.sync.dma_start(acts_d[:], za[:])
            if STAGE >= 2:
                # ---- A_t / G2_t precompute ----
                tmp = smallp.tile([128, F, C], f32, tag="tfc0")
                nc.vector.tensor_scalar_add(tmp[:], u2[:], 0.5)
                rr = smallp.tile([128, F, C], f32, tag="tfc1")
                nc.vector.reciprocal(rr[:], tmp[:])        # 1/(0.5+u2)
                h = smallp.tile([128, F, C], f32, tag="tfc2")
                nc.vector.tensor_mul(h[:], u2[:], rr[:])   # u2/(0.5+u2)
                hsq = smallp.tile([128, F, C], f32, tag="tfc0", name="hsq")
                nc.scalar.square(hsq[:], h[:])
                nc.scalar.activation(A_t[:], hsq[:], AF.Copy, bias=1.0,
                                     scale=-1.0)
                srt = smallp.tile([128, F, C], f32, tag="tfc2", name="srt")
                nc.scalar.activation(srt[:], u2[:], AF.Sqrt, scale=4.0)
                nc.vector.tensor_mul(G2_t[:], srt[:], rr[:])

                # ---- s0 out + AllReduce ----
                s0_sb = smallp.tile([32, D, C], f32, tag="ssb")
                for spt, (r0, r1) in zip(sp, RSPLIT):
                    nc.scalar.activation(s0_sb[:, r0:r1, :],
                                         spt[:, 0:r1 - r0, :], AF.Copy)
                nc.sync.dma_start(cc_in[0][:], s0_sb[:])
                nc.gpsimd.collective_compute(
                    "AllReduce", ADD, replica_groups=[list(range(NCORES))],
                    ins=[cc_in[0].opt()], outs=[cc_out[0].opt()])

                # ---- routing iterations ----
                n_iters = NUM_ITERS if STAGE >= 5 else max(
                    0, min(NUM_ITERS, STAGE - 2))
                for it in range(n_iters):
                    s_rep = smallp.tile([128, D + 1, C], f32, tag="srep",
                                        name=f"s_rep{it}")
                    for g in range(4):
                        if it == 0:
                            nc.sync.dma_start(
                                s_rep[32 * g:32 * g + 32, 0:D, :],
                                cc_out[0][:])
                        else:
                            nc.sync.dma_start(
                                s_rep[32 * g:32 * g + 32, :, :],
                                cc_out[it][:])
                    scd = s_rep[:, 0:D, :]
                    ssq = sqpool.tile([128, D, C], f32, tag="ssq",
                                      name=f"ssq{it}")
                    nc.scalar.square(ssq[:], scd)
                    S2 = smallp.tile([128, C], f32, tag="t20a",
                                     name=f"S2_{it}")
                    nc.vector.tensor_reduce(S2[:], ssq.transpose([0, 2, 1]),
                                            X, ADD)
                    dn = smallp.tile([128, C], f32, tag="t20b",
                                     name=f"dn_{it}")
                    if it == 0:
                        nc.vector.tensor_scalar_add(dn[:], S2[:],
                                                    0.5 * float(N) * float(N))
                    else:
                        cnt = s_rep[:, D:D + 1, :].squeeze(1)
                        k2 = smallp.tile([128, C], f32, tag="t20c",
                                         name=f"k2_{it}")
                        nc.vector.tensor_mul(k2[:], cnt, cnt)
                        nc.vector.scalar_tensor_tensor(
                            dn[:], in0=k2[:], scalar=0.5, in1=S2[:],
                            op0=MULT, op1=ADD)
                    rec = smallp.tile([128, C], f32, tag="t20d",
                                      name=f"rec_{it}")
                    nc.vector.reciprocal(rec[:], dn[:])
                    rt = smallp.tile([128, C], f32, tag="t20e",
                                     name=f"rt_{it}")
                    nc.scalar.sqrt(rt[:], S2[:])
                    qt = smallp.tile([128, C], f32, tag="t20f",
                                     name=f"qt_{it}")
                    nc.vector.tensor_mul(qt[:], rt[:], rec[:])
                    # v = qt * s (squash with count-normalization folded in)

                    if it < NUM_ITERS - 1:
                        with nc.allow_low_precision("bf16 routing"):
                            v_rep = smallp.tile([128, D, C], DT_U, tag="vrep",
                                                name=f"v_rep{it}")
                            nc.vector.tensor_mul(
                                v_rep[:], scd,
                                qt.unsqueeze(1).broadcast_to([128, D, C]))
                            q2 = smallp.tile([128, C], f32, tag="t20c",
                                             name=f"q2_{it}")
                            nc.vector.tensor_mul(q2[:], qt[:], qt[:])
                            vn2 = smallp.tile([128, C], f32, tag="t20g",
                                              name=f"vn2_{it}")
                            nc.vector.tensor_mul(vn2[:], q2[:], S2[:])
                            # dot[p,f,c] = sum_d u_hat * v
                            for ch in range(8):
                                r0 = 4 * ch
                                prod = wpool.tile([128, 4, D, C], DT_U,
                                                  tag="big",
                                                  name=f"pr{it}_{ch}")
                                nc.vector.tensor_mul(
                                    prod[:], uh[:, r0:r0 + 4],
                                    v_rep.unsqueeze(1)
                                    .broadcast_to([128, 4, D, C]))
                                cur = prod
                                w = D
                                while w > 2:
                                    nw = w // 2
                                    tout = treep.tile(
                                        [128, 4, nw, C], DT_U, tag=f"tr{nw}",
                                        name=f"tr{nw}_{it}_{ch}")
                                    nc.vector.tensor_add(
                                        tout[:], cur[:, :, 0:nw, :],
                                        cur[:, :, nw:w, :])
                                    cur = tout
                                    w = nw
                                nc.vector.tensor_add(
                                    dot[:, r0:r0 + 4, :].unsqueeze(2),
                                    cur[:, :, 0:1, :], cur[:, :, 1:2, :])
                        # b_ij += 1 - |usq|^2 + 2g*dot - |v|^2
                        t1 = smallp.tile([128, F, C], f32, tag="tfc0",
                                         name=f"t1_{it}")
                        nc.vector.tensor_mul(t1[:], G2_t[:], dot[:])
                        t2 = smallp.tile([128, F, C], f32, tag="tfc1",
                                         name=f"t2_{it}")
                        nc.vector.tensor_add(t2[:], t1[:], A_t[:])
                        t3 = smallp.tile([128, F, C], f32, tag="tfc2",
                                         name=f"t3_{it}")
                        nc.vector.tensor_sub(
                            t3[:], t2[:],
                            vn2.unsqueeze(1).broadcast_to([128, F, C]))
                        if it == 0:
                            nc.vector.tensor_copy(bij[:], t3[:])
                        else:
                            nc.vector.tensor_add(bij[:], bij[:], t3[:])
                        # softmax over classes -> c (bf16)
                        ex = smallp.tile([128, F, C], f32, tag="tfc0",
                                         name=f"ex_{it}")
                        nc.scalar.activation(ex[:], bij[:], AF.Exp)
                        sm = smallp.tile([128, F], f32, tag="tf0",
                                         name=f"sm_{it}")
                        nc.vector.tensor_reduce(sm[:], ex[:], X, ADD)
                        smr = smallp.tile([128, F], f32, tag="tf1",
                                          name=f"smr_{it}")
                        nc.vector.reciprocal(smr[:], sm[:])
                        c_bf = smallp.tile([128, F, C], DT_U, tag="cbf",
                                           name=f"c_bf{it}")
                        with nc.allow_low_precision("softmax weights bf16"):
                            nc.vector.tensor_mul(
                                c_bf[:], ex[:],
                                smr.broadcast_to([128, F, C]))
                        # next s partials: t = c*u_hat (+count row), PE n-sum
                        spn = [psS.tile([32, 25, C], f32, tag="spA",
                                        name=f"spn_a{it}"),
                               psS.tile([32, 25, C], f32, tag="spB",
                                        name=f"spn_b{it}"),
                               psS.tile([32, 15, C], f32, tag="spC",
                                        name=f"spn_c{it}")]
                        with nc.allow_low_precision("bf16 weighted terms"):
                            for ch in range(8):
                                r0 = 4 * ch
                                tt = wpool.tile([128, 4, D + 1, C], DT_U,
                                                tag="big",
                                                name=f"tt{it}_{ch}")
                                nc.vector.tensor_mul(
                                    tt[:, :, 0:D, :], uh[:, r0:r0 + 4],
                                    c_bf[:, r0:r0 + 4].unsqueeze(2)
                                    .broadcast_to([128, 4, D, C]))
                                nc.vector.tensor_copy(
                                    tt[:, :, D:D + 1, :],
                                    c_bf[:, r0:r0 + 4].unsqueeze(2))
                                for j in range(4):
                                    f = r0 + j
                                    for spt, (r0_, r1_) in zip(spn,
                                                               RSPLIT65):
                                        nc.tensor.matmul(
                                            spt[:, 0:r1_ - r0_, :],
                                            lhsT=blk_sb[:],
                                            rhs=tt[:, j, r0_:r1_, :],
                                            start=(f == 0),
                                            stop=(f == F - 1))
                        sn_sb = smallp.tile([32, D + 1, C], f32, tag="ssb",
                                            name=f"sn_sb{it}")
                        for spt, (r0_, r1_) in zip(spn, RSPLIT65):
                            nc.scalar.activation(sn_sb[:, r0_:r1_, :],
                                                 spt[:, 0:r1_ - r0_, :],
                                                 AF.Copy)
                        nc.sync.dma_start(cc_in[it + 1][:], sn_sb[:])
                        nc.gpsimd.collective_compute(
                            "AllReduce", ADD,
                            replica_groups=[list(range(NCORES))],
                            ins=[cc_in[it + 1].opt()],
                            outs=[cc_out[it + 1].opt()])
                    else:
                        v32 = smallp.tile([32, D, C], f32, tag="v32")
                        nc.vector.tensor_mul(
                            v32[:], scd[0:32],
                            qt[0:32].unsqueeze(1).broadcast_to([32, D, C]))
                        a32 = smallp.tile([32, C], f32, tag="a32o")
                        nc.vector.tensor_mul(a32[:], qt[0:32], rt[0:32])
                        nc.sync.dma_start(poses_d[:], v32[:])
                        nc.sync.dma_start(acts_d[:], a32[:])
                if 2 <= STAGE <= 3:
                    nc.sync.dma_start(dbg_d[:],
                                      A_t[:] if STAGE == 2 else bij[:])
                    z32b = smallp.tile([32, D, C], f32, tag='v32',
                                       name='z32b')
                    nc.vector.tensor_scalar_add(z32b[:], z32b[:], 0.0)
                    zab = smallp.tile([32, C], f32, tag='a32o', name='zab')
                    nc.vector.tensor_scalar_add(zab[:], zab[:], 0.0)
                    nc.sync.dma_start(poses_d[:], z32b[:])
                    nc.sync.dma_start(acts_d[:], zab[:])

    nc.compile()
    return nc


def _get_prog():
    global _prog
    if _prog is None:
        _prog = _build()
    return _prog


def _host_inputs(x, w1):
    import ml_dtypes
    x = np.asarray(x, dtype=np.float32).astype(ml_dtypes.bfloat16)
    w1 = np.asarray(w1, dtype=np.float32).astype(ml_dtypes.bfloat16)
    blk = np.zeros((128, B), dtype=ml_dtypes.bfloat16)
    blk[np.arange(128), np.arange(128) % 32] = 1.0
    in_maps = []
    for r in range(NCORES):
        xs = x[:, r * NL:(r + 1) * NL, :]                 # [B, NL, CI]
        xT = np.ascontiguousarray(xs.transpose(2, 1, 0))  # [CI, NL, B]
        ws = w1[r * NL:(r + 1) * NL]                      # [NL, C, D, CI]
        wT = np.ascontiguousarray(ws.transpose(0, 3, 2, 1))  # [NL, CI, D, C]
        in_maps.append({"xT": xT, "w1t": wT, "blk": blk})
    return in_maps


def kernel(x, w1):
    from concourse.bass_utils import run_bass_kernel_spmd
    nc = _get_prog()
    in_maps = _host_inputs(x, w1)
    res = run_bass_kernel_spmd(nc, in_maps, list(range(NCORES)))
    poses = np.asarray(res.results[0]["poses"],
                       dtype=np.float32).transpose(0, 2, 1).reshape(B, C, D, 1)
    acts = np.asarray(res.results[0]["acts"],
                      dtype=np.float32).reshape(B, C, 1)
    return poses, acts


# revision 9
# speedup vs baseline: 1.2812x; 1.2812x over previous
"""FCCaps (capsule fully-connected layer with dynamic routing) on 8 TRN2 cores.

Sharding: input capsules N=1024 are split 128 per core (w1 is read exactly
once across the chip); batch B=32 is replicated on partitions. Per-core
layout puts partition p = 32*g + b (g = n%4 group, b = batch), free axes
(f = n//4, d, class) -- innermost axis is classes so DVE tensor_tensor ops
keep 2x mode under d-broadcasts. Routing reductions over n run on the PE
via a block-diagonal-ones stationary accumulating into PSUM; the tiny
[B,D(+count),C] partial sums are AllReduced across cores each iteration.
"""
import numpy as np

NCORES = 8
B, N, CI, C, D = 32, 1024, 128, 20, 64
NL = N // NCORES          # 128 local capsules
F, G = 32, 4              # n_local = 4*f + g
CD = C * D                # 1280
NUM_ITERS = 3

_prog = None


def _build():
    import os
    STAGE = int(os.environ.get("KERNEL_STAGE", "99"))
    import concourse.mybir as mybir
    import concourse.bacc as bacc
    import concourse.tile as tile

    f32 = mybir.dt.float32
    bf16 = mybir.dt.bfloat16
    DT_W = bf16     # w1/x dram + sbuf dtype
    DT_U = bf16     # u_hat / t / c / v_rep storage
    X = mybir.AxisListType.X
    ADD = mybir.AluOpType.add
    MULT = mybir.AluOpType.mult
    AF = mybir.ActivationFunctionType

    nc = bacc.Bacc("TRN2", target_bir_lowering=False, debug=False,
                   num_devices=NCORES)

    xT_d = nc.dram_tensor("xT", [CI, NL, B], DT_W, kind="ExternalInput").ap()
    w1t_d = nc.dram_tensor("w1t", [NL, CI, D, C], DT_W,
                           kind="ExternalInput").ap()
    blk_d = nc.dram_tensor("blk", [128, B], DT_U, kind="ExternalInput").ap()
    poses_d = nc.dram_tensor("poses", [B, D, C], f32,
                             kind="ExternalOutput").ap()
    acts_d = nc.dram_tensor("acts", [B, C], f32, kind="ExternalOutput").ap()
    dbg_d = None
    if STAGE < 99:
        dbg_d = nc.dram_tensor("dbg", [128, F, C], f32,
                               kind="ExternalOutput").ap()

    # d-row chunk splits (rows of C=20 fp32 words; PSUM bank = 512 words)
    RSPLIT = [(0, 25), (25, 50), (50, 64)]       # for [*, 64, 20] psum
    RSPLIT65 = [(0, 25), (25, 50), (50, 65)]     # for [*, 65, 20] psum

    with tile.TileContext(nc) as tc:
        with tc.tile_pool(name="persist", bufs=1) as persist, \
             tc.tile_pool(name="wpool", bufs=3) as wpool, \
             tc.tile_pool(name="sqpool", bufs=2) as sqpool, \
             tc.tile_pool(name="treep", bufs=1) as treep, \
             tc.tile_pool(name="smallp", bufs=1) as smallp, \
             tc.tile_pool(name="psA", bufs=1, space="PSUM") as psA, \
             tc.tile_pool(name="psS", bufs=1, space="PSUM") as psS, \
             tc.tile_pool(name="dramp", bufs=1, space="DRAM") as dramp:

            # ---- persistent tiles ----
            uh = persist.tile([128, F, D, C], DT_U)     # u_hat, bf16
            x_sb = persist.tile([CI, NL, B], DT_W)
            blk_sb = persist.tile([128, B], DT_U)
            u2 = persist.tile([128, F, C], f32)         # |u_hat|^2 over d
            A_t = persist.tile([128, F, C], f32)        # 1 - (u2/(.5+u2))^2
            G2_t = persist.tile([128, F, C], f32)       # 2*sqrt(u2)/(.5+u2)
            bij = persist.tile([128, F, C], f32)
            dot = persist.tile([128, F, C], f32)

            # collective buffers
            cc_in = [dramp.tile([B, D, C], f32, name="cc_in0")] + [
                dramp.tile([B, D + 1, C], f32, name=f"cc_in{i}")
                for i in (1, 2)]
            cc_out = [dramp.tile([B, D, C], f32, addr_space="Shared",
                                 name="cc_out0")] + [
                dramp.tile([B, D + 1, C], f32, addr_space="Shared",
                           name=f"cc_out{i}") for i in (1, 2)]

            nc.sync.dma_start(x_sb[:], xT_d[:])
            nc.sync.dma_start(blk_sb[:], blk_d[:])

            # s0 accumulators (c uniform in iter 0 -> plain sum over n)
            sp = [psS.tile([32, 25, C], f32, tag="spA", name="sp_a"),
                  psS.tile([32, 25, C], f32, tag="spB", name="sp_b"),
                  psS.tile([32, 15, C], f32, tag="spC", name="sp_c")]

            # ---- Phase A: u_hat = w1 @ x, norms, incremental s0 ----
            for q in range(F):
                w_t = wpool.tile([CI, G, D, C], DT_W, tag="big",
                                 name=f"w_t{q}")
                nc.sync.dma_start(
                    w_t[:, 0:2], w1t_d[4 * q:4 * q + 2]
                    .rearrange("n i d c -> i n d c"))
                nc.sync.dma_start(
                    w_t[:, 2:4], w1t_d[4 * q + 2:4 * q + 4]
                    .rearrange("n i d c -> i n d c"))
                ph = [psA.tile([128, r1 - r0, C], f32, tag=f"psA{k}",
                               name=f"ph{k}_{q}")
                      for k, (r0, r1) in enumerate(RSPLIT)]
                for g in range(G):
                    lhsT = x_sb[:, 4 * q + g, :]
                    for k, (r0, r1) in enumerate(RSPLIT):
                        nc.tensor.matmul(
                            ph[k][32 * g:32 * g + 32, :, :],
                            lhsT=lhsT,
                            rhs=w_t[:, g, r0:r1, :],
                            start=True, stop=True,
                            tile_position=(0, 32 * g))
                sq = sqpool.tile([128, D, C], DT_U, tag="sq", name=f"sq{q}")
                for k, (r0, r1) in enumerate(RSPLIT):
                    nc.scalar.activation(uh[:, q, r0:r1, :], ph[k][:],
                                         AF.Copy)
                    nc.scalar.square(sq[:, r0:r1, :], ph[k][:])
                with nc.allow_low_precision("bf16 u2 tree"):
                    cur = sq
                    w = D
                    while w > 2:
                        nw = w // 2
                        tout = treep.tile([128, nw, C], DT_U, tag=f"us{nw}",
                                          name=f"us{nw}_{q}")
                        nc.vector.tensor_add(tout[:], cur[:, 0:nw, :],
                                             cur[:, nw:w, :])
                        cur = tout
                        w = nw
                    nc.vector.tensor_add(u2[:, q].unsqueeze(1),
                                         cur[:, 0:1, :], cur[:, 1:2, :])
                for spt, (r0, r1) in zip(sp, RSPLIT):
                    nc.tensor.matmul(spt[:, 0:r1 - r0, :], lhsT=blk_sb[:],
                                     rhs=uh[:, q, r0:r1, :],
                                     start=(q == 0), stop=(q == F - 1))

            if STAGE <= 1:
                nc.sync.dma_start(dbg_d[:], u2[:])
                z32 = smallp.tile([32, D, C], f32, tag="v32")
                nc.vector.tensor_scalar_add(z32[:], z32[:], 0.0)
                za = smallp.tile([32, C], f32, tag="a32o")
                nc.vector.tensor_scalar_add(za[:], za[:], 0.0)
                nc.sync.dma_start(poses_d[:], z32[:])
                nc.sync.dma_start(acts_d[:], za[:])
            if STAGE >= 2:
                # ---- A_t / G2_t precompute ----
                tmp = smallp.tile([128, F, C], f32, tag="tfc0")
                nc.vector.tensor_scalar_add(tmp[:], u2[:], 0.5)
                rr = smallp.tile([128, F, C], f32, tag="tfc1")
                nc.vector.reciprocal(rr[:], tmp[:])        # 1/(0.5+u2)
                h = smallp.tile([128, F, C], f32, tag="tfc2")
                nc.vector.tensor_mul(h[:], u2[:], rr[:])   # u2/(0.5+u2)
                hsq = smallp.tile([128, F, C], f32, tag="tfc0", name="hsq")
                nc.scalar.square(hsq[:], h[:])
                nc.scalar.activation(A_t[:], hsq[:], AF.Copy, bias=1.0,
                                     scale=-1.0)
                srt = smallp.tile([128, F, C], f32, tag="tfc2", name="srt")
                nc.scalar.activation(srt[:], u2[:], AF.Sqrt, scale=4.0)
                nc.vector.tensor_mul(G2_t[:], srt[:], rr[:])

                # ---- s0 out + AllReduce ----
                s0_sb = smallp.tile([32, D, C], f32, tag="ssb")
                for spt, (r0, r1) in zip(sp, RSPLIT):
                    nc.scalar.activation(s0_sb[:, r0:r1, :],
                                         spt[:, 0:r1 - r0, :], AF.Copy)
                nc.sync.dma_start(cc_in[0][:], s0_sb[:])
                nc.gpsimd.collective_compute(
                    "AllReduce", ADD, replica_groups=[list(range(NCORES))],
                    ins=[cc_in[0].opt()], outs=[cc_out[0].opt()])

                # ---- routing iterations ----
                n_iters = NUM_ITERS if STAGE >= 5 else max(
                    0, min(NUM_ITERS, STAGE - 2))
                for it in range(n_iters):
                    s_rep = smallp.tile([128, D + 1, C], f32, tag="srep",
                                        name=f"s_rep{it}")
                    for g in range(4):
                        if it == 0:
                            nc.sync.dma_start(
                                s_rep[32 * g:32 * g + 32, 0:D, :],
                                cc_out[0][:])
                        else:
                            nc.sync.dma_start(
                                s_rep[32 * g:32 * g + 32, :, :],
                                cc_out[it][:])
                    scd = s_rep[:, 0:D, :]
                    ssq = sqpool.tile([128, D, C], f32, tag="ssq",
                                      name=f"ssq{it}")
                    nc.scalar.square(ssq[:], scd)
                    S2 = smallp.tile([128, C], f32, tag="t20a",
                                     name=f"S2_{it}")
                    nc.vector.tensor_reduce(S2[:], ssq.transpose([0, 2, 1]),
                                            X, ADD)
                    dn = smallp.tile([128, C], f32, tag="t20b",
                                     name=f"dn_{it}")
                    if it == 0:
                        nc.vector.tensor_scalar_add(dn[:], S2[:],
                                                    0.5 * float(N) * float(N))
                    else:
                        cnt = s_rep[:, D:D + 1, :].squeeze(1)
                        k2 = smallp.tile([128, C], f32, tag="t20c",
                                         name=f"k2_{it}")
                        nc.vector.tensor_mul(k2[:], cnt, cnt)
                        nc.vector.scalar_tensor_tensor(
                            dn[:], in0=k2[:], scalar=0.5, in1=S2[:],
                            op0=MULT, op1=ADD)
                    rec = smallp.tile([128, C], f32, tag="t20d",
                                      name=f"rec_{it}")
                    nc.vector.reciprocal(rec[:], dn[:])
                    rt = smallp.tile([128, C], f32, tag="t20e",
                                     name=f"rt_{it}")
                    nc.scalar.sqrt(rt[:], S2[:])
                    qt = smallp.tile([128, C], f32, tag="t20f",
                                     name=f"qt_{it}")
                    nc.vector.tensor_mul(qt[:], rt[:], rec[:])
                    # v = qt * s (squash with count-normalization folded in)

                    if it < NUM_ITERS - 1:
                        with nc.allow_low_precision("bf16 routing"):
                            v_rep = smallp.tile([128, D, C], DT_U, tag="vrep",
                                                name=f"v_rep{it}")
                            nc.vector.tensor_mul(
                                v_rep[:], scd,
                                qt.unsqueeze(1).broadcast_to([128, D, C]))
                            q2 = smallp.tile([128, C], f32, tag="t20c",
                                             name=f"q2_{it}")
                            nc.vector.tensor_mul(q2[:], qt[:], qt[:])
                            vn2 = smallp.tile([128, C], f32, tag="t20g",
                                              name=f"vn2_{it}")
                            nc.vector.tensor_mul(vn2[:], q2[:], S2[:])
                            # dot[p,f,c] = sum_d u_hat * v
                            for ch in range(8):
                                r0 = 4 * ch
                                prod = wpool.tile([128, 4, D, C], DT_U,
                                                  tag="big",
                                                  name=f"pr{it}_{ch}")
                                nc.vector.tensor_mul(
                                    prod[:], uh[:, r0:r0 + 4],
                                    v_rep.unsqueeze(1)
                                    .broadcast_to([128, 4, D, C]))
                                cur = prod
                                w = D
                                while w > 2:
                                    nw = w // 2
                                    tout = treep.tile(
                                        [128, 4, nw, C], DT_U, tag=f"tr{nw}",
                                        name=f"tr{nw}_{it}_{ch}")
                                    nc.vector.tensor_add(
                                        tout[:], cur[:, :, 0:nw, :],
                                        cur[:, :, nw:w, :])
                                    cur = tout
                                    w = nw
                                nc.vector.tensor_add(
                                    dot[:, r0:r0 + 4, :].unsqueeze(2),
                                    cur[:, :, 0:1, :], cur[:, :, 1:2, :])
                        # b_ij += 1 - |usq|^2 + 2g*dot - |v|^2
                        t1 = smallp.tile([128, F, C], f32, tag="tfc0",
                                         name=f"t1_{it}")
                        nc.vector.tensor_mul(t1[:], G2_t[:], dot[:])
                        t2 = smallp.tile([128, F, C], f32, tag="tfc1",
                                         name=f"t2_{it}")
                        nc.vector.tensor_add(t2[:], t1[:], A_t[:])
                        t3 = smallp.tile([128, F, C], f32, tag="tfc2",
                                         name=f"t3_{it}")
                        nc.vector.tensor_sub(
                            t3[:], t2[:],
                            vn2.unsqueeze(1).broadcast_to([128, F, C]))
                        if it == 0:
                            nc.vector.tensor_copy(bij[:], t3[:])
                        else:
                            nc.vector.tensor_add(bij[:], bij[:], t3[:])
                        # softmax over classes -> c (bf16)
                        ex = smallp.tile([128, F, C], f32, tag="tfc0",
                                         name=f"ex_{it}")
                        nc.scalar.activation(ex[:], bij[:], AF.Exp)
                        sm = smallp.tile([128, F], f32, tag="tf0",
                                         name=f"sm_{it}")
                        nc.vector.tensor_reduce(sm[:], ex[:], X, ADD)
                        smr = smallp.tile([128, F], f32, tag="tf1",
                                          name=f"smr_{it}")
                        nc.vector.reciprocal(smr[:], sm[:])
                        c_bf = smallp.tile([128, F, C], DT_U, tag="cbf",
                                           name=f"c_bf{it}")
                        with nc.allow_low_precision("softmax weights bf16"):
                            nc.vector.tensor_mul(
                                c_bf[:], ex[:],
                                smr.broadcast_to([128, F, C]))
                        # next s partials: t = c*u_hat (+count row), PE n-sum
                        spn = [psS.tile([32, 25, C], f32, tag="spA",
                                        name=f"spn_a{it}"),
                               psS.tile([32, 25, C], f32, tag="spB",
                                        name=f"spn_b{it}"),
                               psS.tile([32, 15, C], f32, tag="spC",
                                        name=f"spn_c{it}")]
                        with nc.allow_low_precision("bf16 weighted terms"):
                            for ch in range(8):
                                r0 = 4 * ch
                                tt = wpool.tile([128, 4, D + 1, C], DT_U,
                                                tag="big",
                                                name=f"tt{it}_{ch}")
                                nc.vector.tensor_mul(
                                    tt[:, :, 0:D, :], uh[:, r0:r0 + 4],
                                    c_bf[:, r0:r0 + 4].unsqueeze(2)
                                    .broadcast_to([128, 4, D, C]))
                                nc.vector.tensor_copy(
                                    tt[:, :, D:D + 1, :],
                                    c_bf[:, r0:r0 + 4].unsqueeze(2))
                                for j in range(4):
                                    f = r0 + j
                                    for spt, (r0_, r1_) in zip(spn,
                                                               RSPLIT65):
                                        nc.tensor.matmul(
                                            spt[:, 0:r1_ - r0_, :],
                                            lhsT=blk_sb[:],
                                            rhs=tt[:, j, r0_:r1_, :],
                                            start=(f == 0),
                                            stop=(f == F - 1))
                        sn_sb = smallp.tile([32, D + 1, C], f32, tag="ssb",
                                            name=f"sn_sb{it}")
                        for spt, (r0_, r1_) in zip(spn, RSPLIT65):
                            nc.scalar.activation(sn_sb[:, r0_:r1_, :],
                                                 spt[:, 0:r1_ - r0_, :],
                                                 AF.Copy)
                        nc.sync.dma_start(cc_in[it + 1][:], sn_sb[:])
                        nc.gpsimd.collective_compute(
                            "AllReduce", ADD,
                            replica_groups=[list(range(NCORES))],
                            ins=[cc_in[it + 1].opt()],
                            outs=[cc_out[it + 1].opt()])
                    else:
                        v32 = smallp.tile([32, D, C], f32, tag="v32")
                        nc.vector.tensor_mul(
                            v32[:], scd[0:32],
                            qt[0:32].unsqueeze(1).broadcast_to([32, D, C]))
                        a32 = smallp.tile([32, C], f32, tag="a32o")
                        nc.vector.tensor_mul(a32[:], qt[0:32], rt[0:32])
                        nc.sync.dma_start(poses_d[:], v32[:])
                        nc.sync.dma_start(acts_d[:], a32[:])
                if 2 <= STAGE <= 3:
                    nc.sync.dma_start(dbg_d[:],
                                      A_t[:] if STAGE == 2 else bij[:])
                    z32b = smallp.tile([32, D, C], f32, tag='v32',
                                       name='z32b')
                    nc.vector.tensor_scalar_add(z32b[:], z32b[:], 0.0)
                    zab = smallp.tile([32, C], f32, tag='a32o', name='zab')
                    nc.vector.tensor_scalar_add(zab[:], zab[:], 0.0)
                    nc.sync.dma_start(poses_d[:], z32b[:])
                    nc.sync.dma_start(acts_d[:], zab[:])

    nc.compile()
    return nc


def _get_prog():
    global _prog
    if _prog is None:
        _prog = _build()
    return _prog


def _host_inputs(x, w1):
    import ml_dtypes
    x = np.asarray(x, dtype=np.float32).astype(ml_dtypes.bfloat16)
    w1 = np.asarray(w1, dtype=np.float32).astype(ml_dtypes.bfloat16)
    blk = np.zeros((128, B), dtype=ml_dtypes.bfloat16)
    blk[np.arange(128), np.arange(128) % 32] = 1.0
    in_maps = []
    for r in range(NCORES):
        xs = x[:, r * NL:(r + 1) * NL, :]                 # [B, NL, CI]
        xT = np.ascontiguousarray(xs.transpose(2, 1, 0))  # [CI, NL, B]
        ws = w1[r * NL:(r + 1) * NL]                      # [NL, C, D, CI]
        wT = np.ascontiguousarray(ws.transpose(0, 3, 2, 1))  # [NL, CI, D, C]
        in_maps.append({"xT": xT, "w1t": wT, "blk": blk})
    return in_maps


def kernel(x, w1):
    from concourse.bass_utils import run_bass_kernel_spmd
    nc = _get_prog()
    in_maps = _host_inputs(x, w1)
    res = run_bass_kernel_spmd(nc, in_maps, list(range(NCORES)))
    poses = np.asarray(res.results[0]["poses"],
                       dtype=np.float32).transpose(0, 2, 1).reshape(B, C, D, 1)
    acts = np.asarray(res.results[0]["acts"],
                      dtype=np.float32).reshape(B, C, 1)
    return poses, acts
